# revision 28
# baseline (speedup 1.0000x reference)
"""MDCA loss kernel for Trainium2, 8 NeuronCores, data-parallel over batch.

reference:
    counts[c]   = histogram(target) ; avg_count = counts/B
    avg_conf    = mean(logits, axis=1)            # [E, C]
    loss[e]     = mean_c |avg_conf[e,c] - avg_count[c]|

Strategy per core (batch shard of 1024 rows, partition p holds rows 8p..8p+7):
  - per exit, two row-half DMAs [128p, 4i*1000c] — each partition line is one
    contiguous 16KB read; 8 big DMAs spread over the sync/scalar/gpsimd DGE
    queues (~330 GB/s aggregate, wire-bound)
  - DVE folds each half 4->2 row-groups with one contiguous tensor_add
    (output in float32r)
  - float32r PE matmuls (1 cyc/row) with a [128,4] selector (ones in column
    e) fold the remaining row-groups AND the 128 partitions into PSUM[4,500]
    per C-half, accumulating conf_e sums across exits via start/stop chains
  - histogram: iota + tensor_scalar(is_equal) one-hots (bf16, on DVE while
    the first DMAs are in flight), matmul with [128,4] -1 weights ->
    PSUM[4,500] accumulates -count broadcast to all 4 exit rows
  - part[4, 1000] = psum_conf + psum_hist = conf_e,c - cnt_c   (per-shard)
  - host sums the 8 per-core partials and takes |.|-mean / (B*C) -> loss[4]
    (DEVICE_FINISH=True instead runs an on-device AllReduce + abs/mean, but
    the 16KB collective costs ~35us on HW, so host finish wins)
"""

import os
import sys

for _p in ("/opt/trn_rl_repo", "/root/.axon_site/_ro/trn_rl_repo"):
    if os.path.isdir(_p) and _p not in sys.path:
        sys.path.insert(0, _p)

import numpy as np

import concourse.bass as bass
import concourse.bacc as bacc
import concourse.tile as tile
import concourse.mybir as mybir
from concourse.bass_utils import run_bass_kernel_spmd

E, B, C = 4, 8192, 1000
N_CORES = 8
BS = B // N_CORES          # 1024 batch rows per core
GP = 8                     # rows folded per partition (BS = 128 * GP)
CH = C // 2                # 500, C half per PSUM bank
F32 = mybir.dt.float32
BF16 = mybir.dt.bfloat16

# True: on-device AllReduce + final abs/mean, output "loss" [4].
# False: per-core partials "part" [4, C]; host sums cores and finishes.
DEVICE_FINISH = False


def build_nc(device_finish: bool = DEVICE_FINISH):
    nc = bacc.Bacc(
        "TRN2",
        target_bir_lowering=False,
        debug=False,
        enable_asserts=False,
        num_devices=N_CORES,
    )

    logits = nc.dram_tensor("logits", [E, BS, C], F32, kind="ExternalInput")
    # host pre-arranges the target shard as [128, GP] float32 (exact ints)
    target = nc.dram_tensor("target_f", [128, GP], F32, kind="ExternalInput")
    if device_finish:
        loss = nc.dram_tensor("loss", [E, 1], F32, kind="ExternalOutput")
        cc_in = nc.dram_tensor("cc_in", [E, C], F32)
        cc_out = nc.dram_tensor("cc_out", [E, C], F32, addr_space="Shared")
    else:
        part_out = nc.dram_tensor("part", [E, C], F32, kind="ExternalOutput")

    with tile.TileContext(nc) as tc:
        with (
            tc.tile_pool(name="const", bufs=1) as const,
            tc.tile_pool(name="ld", bufs=8) as ld,
            tc.tile_pool(name="work", bufs=3) as work,
            tc.tile_pool(name="psum", bufs=1, space=bass.MemorySpace.PSUM) as psum,
        ):
            # selector weights: sels[:, 4e:4e+4] has ones in column e
            # (float32r so the PE folds run at 1 cyc/row instead of 4;
            # memset can't write f32r, so build in f32 and convert)
            sels_f = const.tile([128, 4 * E], F32, tag="sels_f")
            nc.vector.memset(sels_f[:], 0.0)
            for e in range(E):
                nc.vector.memset(sels_f[:, 4 * e + e : 4 * e + e + 1], 1.0)
            sels = const.tile([128, 4 * E], mybir.dt.float32r, tag="sels")
            nc.vector.tensor_copy(sels[:], sels_f[:])
            neg1 = const.tile([128, E], BF16, tag="neg1")
            nc.vector.memset(neg1[:], -1.0)
            iota_f = const.tile([128, C], F32, tag="iota")
            nc.gpsimd.iota(
                iota_f[:],
                pattern=[[1, C]],
                base=0,
                channel_multiplier=0,
                allow_small_or_imprecise_dtypes=True,
            )
            tgt_sb = const.tile([128, GP], F32, tag="tgt")
            nc.gpsimd.dma_start(out=tgt_sb[:], in_=target[:])

            psum_conf = [
                psum.tile([E, CH], F32, tag=f"pconf{h}", name=f"pconf{h}")
                for h in range(2)
            ]
            psum_hist = [
                psum.tile([E, CH], F32, tag=f"phist{h}", name=f"phist{h}")
                for h in range(2)
            ]

            # histogram one-hots first: they only need iota+targets, so the
            # DVE does them while the first logits DMAs are still in flight
            F32R = mybir.dt.float32r
            for i in range(GP):
                onehot = work.tile([128, C], BF16, tag="onehot")
                nc.vector.tensor_scalar(
                    onehot[:],
                    iota_f[:],
                    tgt_sb[:, i : i + 1],
                    None,
                    mybir.AluOpType.is_equal,
                )
                for hh in range(2):
                    nc.tensor.matmul(
                        psum_hist[hh][:],
                        neg1[:],
                        onehot[:, hh * CH : (hh + 1) * CH],
                        start=(i == 0),
                        stop=(i == GP - 1),
                    )

            # main reduction: per exit, two row-half DMAs (rows i0-3 / i4-7 of
            # each partition's 8) — per-partition source is one contiguous
            # 16KB line, the best possible DMA pattern. DVE folds each half
            # 4->2 row-groups with one contiguous tensor_add; float32r
            # matmuls (1 cyc/row) fold the remaining groups and the 128
            # partitions into PSUM[4, CH].
            dma_engines = [nc.sync, nc.scalar, nc.gpsimd]
            hg = GP // 2  # row-groups per half-DMA
            for e in range(E):
                src = logits[e].rearrange("(p i) c -> p i c", i=GP)
                for x in range(2):
                    t = ld.tile([128, hg * C], F32, tag="ldt", name=f"ldt{e}_{x}")
                    dma_engines[(2 * e + x) % 3].dma_start(
                        out=t.rearrange("p (i c) -> p i c", i=hg),
                        in_=src[:, x * hg : (x + 1) * hg, :],
                    )
                    f = work.tile([128, 2 * C], F32R, tag="fx", name=f"fx{e}_{x}")
                    nc.vector.tensor_add(f[:], t[:, : 2 * C], t[:, 2 * C :])
                    for g in range(2):
                        for h in range(2):
                            nc.tensor.matmul(
                                psum_conf[h][:],
                                sels[:, 4 * e : 4 * e + 4],
                                f[:, g * C + h * CH : g * C + (h + 1) * CH],
                                start=(e == 0 and x == 0 and g == 0),
                                stop=(e == E - 1 and x == 1 and g == 1),
                            )

            # combine conf + (-count) partials -> [4, C] in SBUF
            # (DVE can read only one PSUM operand per instruction)
            part_sb = work.tile([E, C], F32, tag="part")
            for h in range(2):
                hist_sb = work.tile([E, CH], F32, tag="hist_sb")
                nc.vector.tensor_copy(hist_sb[:], psum_hist[h][:])
                nc.vector.tensor_add(
                    part_sb[:, h * CH : (h + 1) * CH],
                    psum_conf[h][:],
                    hist_sb[:],
                )

            if device_finish:
                nc.sync.dma_start(out=cc_in[:], in_=part_sb[:])
                nc.gpsimd.collective_compute(
                    "AllReduce",
                    mybir.AluOpType.add,
                    replica_groups=[list(range(N_CORES))],
                    ins=[cc_in[:].opt()],
                    outs=[cc_out[:].opt()],
                )
                red_sb = work.tile([E, C], F32, tag="red")
                nc.sync.dma_start(out=red_sb[:], in_=cc_out[:])
                lsum = work.tile([E, 1], F32, tag="lsum")
                nc.vector.reduce_sum(
                    lsum[:],
                    red_sb[:],
                    axis=mybir.AxisListType.X,
                    apply_absolute_value=True,
                )
                lout = work.tile([E, 1], F32, tag="lout")
                nc.vector.tensor_scalar_mul(lout[:], lsum[:], 1.0 / (B * C))
                nc.sync.dma_start(out=loss[:], in_=lout[:])
            else:
                nc.sync.dma_start(out=part_out[:], in_=part_sb[:])

    nc.compile()
    return nc


_NC_CACHE = {}


def _get_nc(device_finish: bool):
    key = device_finish
    if key not in _NC_CACHE:
        _NC_CACHE[key] = build_nc(device_finish)
    return _NC_CACHE[key]


def make_in_maps(logits: np.ndarray, target: np.ndarray):
    logits = np.ascontiguousarray(logits, dtype=np.float32)
    target = np.asarray(target)
    in_maps = []
    for c in range(N_CORES):
        lg = logits[:, c * BS : (c + 1) * BS, :]
        tg = target[c * BS : (c + 1) * BS].astype(np.float32).reshape(128, GP)
        in_maps.append({"logits": np.ascontiguousarray(lg), "target_f": tg})
    return in_maps


def kernel(logits: np.ndarray, target: np.ndarray) -> np.ndarray:
    nc = _get_nc(DEVICE_FINISH)
    in_maps = make_in_maps(logits, target)
    res = run_bass_kernel_spmd(nc, in_maps, core_ids=list(range(N_CORES)))
    if DEVICE_FINISH:
        return np.asarray(res.results[0]["loss"], dtype=np.float32).reshape(E)
    parts = sum(np.asarray(r["part"], dtype=np.float64) for r in res.results)
    return (np.abs(parts).sum(axis=1) / (B * C)).astype(np.float32)
